# revision 65
# baseline (speedup 1.0000x reference)
"""AttentionBlockWithSkipConnection Trainium2 kernel (fp8 DoubleRow version).

Full inputs -> full output. Data-parallel over batch B=8 across 8 cores.
Each core computes one batch: GroupNorm -> qkv 1x1conv -> full 4096x4096
attention -> proj 1x1conv -> skip add.

All heavy matmuls run as fp8e4m3 DoubleRow (256-deep contraction per
instruction at 1 cycle/output-row on hardware -- 2x the fp32r rate). The
softmax denominator is accumulated on the PE by a ones-matmul per k-pair
(DVE adds were the #2 engine cost in the fp32r version), and exp is the
only phase-D op on the ACT engine. Q/K accuracy is recovered by a 2-term
qkv matmul with an fp8 WEIGHT residual (wq8 + wlo8), which removes the
weight-quantization noise for a one-time cost instead of a per-logits one.

Scaling scheme (all powers of two, exact in fp8):
  x8   = fp8(x)                      GN stats computed from x8 (negligible
                                     error, self-consistent)
  wq8  = fp8(8 * w_qkv * a)          a = gn_scale * rstd folded per channel
  wlo8 = fp8(8 * w_qkv * a - wq8)    fp8 residual, 2nd qkv matmul term
  q8,k8,v8 = fp8(8*(qkv))            |8q| < 50 << 240 (e4m3 max)
  logits_psum = q8.k8 = 64 * qk
  exp' = exp(logits/16 - 4 ln2)      via ACT scale=1/1024, bias=-4ln2
                                     exp' max ~117 < 240 (e4m3 max)
  cs   = sum_k 8 * exp'              PE ones(8.0)-matmul, PSUM accumulate
  o_un = av + (cs/8)*bias2_v         fp32, V bias folded via the colsum
  pj   = wp.o_un (fp32r); out = pj * recip(cs) + b_proj + x
         [exact: av = 8*sum exp' v, so pj*1/(8 sum exp') = wp.o]
"""

import numpy as np
import ml_dtypes

import concourse.bacc as bacc
import concourse.mybir as mybir
import concourse.tile as tile

N_CORES = 8
B, H, W, C = 8, 64, 64, 256
N = H * W  # 4096 tokens
G = 32  # groups
GS = C // G  # 8 channels per group
EPS = 1e-5
CC = C // 128  # 2 channel chunks
QT = 512  # q tile (free dim of logits/attnv matmuls)
NQ = N // QT  # 8
NK = N // 128  # 32 k tiles
NPAIR = NK // 2  # 16 k-pairs per q tile
F32 = mybir.dt.float32
F8 = mybir.dt.float8e4
BF16 = mybir.dt.bfloat16
DR = mybir.MatmulPerfMode.DoubleRow

WS = 8.0  # fp8 scale on qkv weights and qkv outputs
# colsum lhsT value: pjm = pj_psum*recip = (WS*sum exp' * pj)/(u*sum exp')
# is exact when u = WS (proj runs in fp32r on the raw av accumulators)
ONES_VAL = WS
EXP_BIAS = -4.0 * float(np.log(2.0))  # keeps exp' < 2^-4 * e^7.6 ~ 117
F32R = mybir.dt.float32r


def _build(repeat=1):
    nc = bacc.Bacc(
        "TRN2",
        target_bir_lowering=False,
        debug=False,
        enable_asserts=True,
        num_devices=N_CORES,
    )
    x_d = nc.dram_tensor("x", [N, C], F32, kind="ExternalInput")
    gns_d = nc.dram_tensor("gn_scale", [C], F32, kind="ExternalInput")
    gnb_d = nc.dram_tensor("gn_bias", [C], F32, kind="ExternalInput")
    wq_d = nc.dram_tensor("w_qkv", [C, 3 * C], F32, kind="ExternalInput")
    bq_d = nc.dram_tensor("b_qkv", [3 * C], F32, kind="ExternalInput")
    wp_d = nc.dram_tensor("w_proj", [C, C], F32, kind="ExternalInput")
    bp_d = nc.dram_tensor("b_proj", [C], F32, kind="ExternalInput")
    out_d = nc.dram_tensor("out", [N, C], F32, kind="ExternalOutput")

    # group-aggregation masks: gA averages 8 consecutive partitions into one
    # group row; gB broadcasts group rows back to their 128 channels.
    gA_np = np.zeros((128, 16), np.float32)
    gB_np = np.zeros((16, 128), np.float32)
    for p in range(128):
        gA_np[p, p // GS] = 1.0 / GS
        gB_np[p // GS, p] = 1.0
    gA_d = nc.inline_tensor(gA_np, "gA")
    gB_d = nc.inline_tensor(gB_np, "gB")
    ident_d = nc.inline_tensor(np.eye(128, dtype=np.float32), "ident")
    ident16_d = nc.inline_tensor(
        np.eye(128).astype(ml_dtypes.bfloat16), "ident16"
    )
    ones8_d = nc.inline_tensor(
        np.full((128, 2 * 128), ONES_VAL, ml_dtypes.float8_e4m3), "ones8"
    )
    ones1_d = nc.inline_tensor(np.ones((1, 128), np.float32), "ones1")

    with tile.TileContext(nc) as tc:
        for _ in range(repeat):
            _body(tc, x_d, gns_d, gnb_d, wq_d, bq_d, wp_d, bp_d, out_d,
                  gA_d, gB_d, ident_d, ident16_d, ones8_d, ones1_d)
    nc.compile()
    return nc


def _body(tc, x_d, gns_d, gnb_d, wq_d, bq_d, wp_d, bp_d, out_d,
          gA_d, gB_d, ident_d, ident16_d, ones8_d, ones1_d):
    nc = tc.nc
    x_tok = x_d.ap().rearrange("(p nt) c -> p nt c", p=128)  # [128, 32, 256]
    out_tok = out_d.ap().rearrange("(p nt) c -> p nt c", p=128)

    with (
        tc.tile_pool(name="consts", bufs=1) as consts,
        tc.tile_pool(name="psum_tr", bufs=1, space="PSUM") as psum_tr,
        tc.tile_pool(name="psum_big", bufs=2, space="PSUM") as psum_big,
        tc.tile_pool(name="psum_acc", bufs=1, space="PSUM") as psum_acc,
        tc.tile_pool(name="xtm", bufs=1) as xtm_pool,
        tc.tile_pool(name="qkv8p", bufs=1) as qkv8_pool,
        tc.tile_pool(name="vtm8p", bufs=1) as vtm8_pool,
        tc.tile_pool(name="work", bufs=2) as work,
        tc.tile_pool(name="expp", bufs=4) as expp,
    ):
        # ---- input DMAs: ident first (gates transposes), then the 4MB of x
        # (its arrival is the phase-A critical path), then small constants,
        # then the big weight stages ----
        ident = consts.tile([128, 128], F32)
        nc.sync.dma_start(out=ident, in_=ident_d.ap())
        x_tm = xtm_pool.tile([128, 32, C], F32)  # 32KB/partition
        dma_engs = [nc.sync, nc.scalar]
        for dchunk in range(16):
            dma_engs[dchunk % 2].dma_start(
                out=x_tm[:, dchunk * 2 : (dchunk + 1) * 2, :],
                in_=x_tok[:, dchunk * 2 : (dchunk + 1) * 2, :],
            )
        gA = consts.tile([128, 16], F32)
        nc.sync.dma_start(out=gA, in_=gA_d.ap())
        gB = consts.tile([16, 128], F32)
        nc.sync.dma_start(out=gB, in_=gB_d.ap())
        ident16 = consts.tile([128, 128], BF16)
        nc.sync.dma_start(out=ident16, in_=ident16_d.ap())
        ones8 = consts.tile([128, 2, 128], F8)
        nc.sync.dma_start(
            out=ones8, in_=ones8_d.ap().rearrange("p (g m) -> p g m", g=2)
        )
        ones1 = consts.tile([1, 128], F32)
        nc.sync.dma_start(out=ones1, in_=ones1_d.ap())
        bq = consts.tile([128, 6], F32)
        nc.sync.dma_start(
            out=bq, in_=bq_d.ap().rearrange("(m p) -> p m", p=128)
        )
        bq8 = consts.tile([128, 6], F32)
        nc.vector.tensor_scalar_mul(out=bq8, in0=bq, scalar1=WS)
        bp_row = consts.tile([1, C], F32)
        nc.sync.dma_start(
            out=bp_row, in_=bp_d.ap().rearrange("(o c) -> o c", o=1)
        )
        gns = consts.tile([128, CC], F32)
        nc.sync.dma_start(
            out=gns, in_=gns_d.ap().rearrange("(cc p) -> p cc", p=128)
        )
        gnb = consts.tile([128, CC], F32)
        nc.sync.dma_start(
            out=gnb, in_=gnb_d.ap().rearrange("(cc p) -> p cc", p=128)
        )
        wq_stage = consts.tile([128, CC, 3 * C], F32)
        nc.sync.dma_start(
            out=wq_stage, in_=wq_d.ap().rearrange("(cc p) d -> p cc d", p=128)
        )
        wp_stage = consts.tile([128, CC, C], F32)
        nc.sync.dma_start(
            out=wp_stage, in_=wp_d.ap().rearrange("(cc p) d -> p cc d", p=128)
        )
        eps_col = consts.tile([128, 1], F32)
        nc.vector.memset(eps_col, EPS)
        ebias = consts.tile([128, 1], F32)
        nc.vector.memset(ebias, EXP_BIAS)
        # dummy sqrt as the ACT engine's first instruction: the sqrt act
        # table loads at t=0 (Copy lives in every table, so the phase-A
        # drains don't force a reload; the real GN sqrt then runs load-free)
        scratch = consts.tile([128, 1], F32)
        nc.scalar.activation(
            out=scratch, in_=eps_col,
            func=mybir.ActivationFunctionType.Sqrt, bias=eps_col,
        )

        # proj weights stay fp32r (cheap: 4 matmuls/qt) to keep the proj
        # stage's quantization out of the error budget
        wp = consts.tile([128, CC, C], F32)
        nc.vector.tensor_copy(out=wp.bitcast(F32R), in_=wp_stage)

        # ---- phase A: transpose x to channel-major fp8; bn_stats on the fp8
        # values, interleaved per 512-token span. 8 transposes pack into one
        # 2-bank pair slot, so the ring holds 16 outstanding transposes. ----
        x_cm8 = consts.tile([128, CC, N], F8)  # 8KB/partition
        stats = work.tile([128, CC, 8, 6], F32, tag="stats", bufs=1)
        for s in range(8):
            # slot layout: [cc0 transposes of nt 4s..4s+3 | cc1 of the same],
            # so each half drains with ONE wide [128,512] copy
            slot = psum_big.tile([128, 1024], F32, tag="pair", name="trA")
            for t in (0, 1, 4, 5, 2, 3, 6, 7):  # chase DMA chunk arrivals
                cc = t // 4
                nt = 4 * s + (t % 4)
                nc.tensor.transpose(
                    slot[:, t * 128 : (t + 1) * 128],
                    x_tm[:, nt, cc * 128 : (cc + 1) * 128], ident,
                )
            nc.scalar.copy(
                out=x_cm8[:, 0, s * 512 : (s + 1) * 512], in_=slot[:, 0:512]
            )
            nc.vector.tensor_copy(
                out=x_cm8[:, 1, s * 512 : (s + 1) * 512], in_=slot[:, 512:1024]
            )
            for cc in range(CC):
                nc.vector.bn_stats(
                    out=stats[:, cc, s, :],
                    in_=x_cm8[:, cc, s * 512 : (s + 1) * 512],
                )

        # ---- groupnorm stats -> per-channel affine (a, b), both channel
        # chunks batched through single psum round-trips ----
        ab = work.tile([128, CC, 2], F32, tag="ab", bufs=1)  # (8a, b)
        mv2a = work.tile([128, CC, 2], F32, tag="mv2a", bufs=1)  # (mean, Ex2)
        for cc in range(CC):
            mv = work.tile([128, 2], F32, tag="mv")
            nc.vector.bn_aggr(out=mv, in_=stats[:, cc, :, :])
            nc.vector.tensor_copy(out=mv2a[:, cc, 0:1], in_=mv[:, 0:1])
            nc.vector.tensor_mul(
                out=mv2a[:, cc, 1:2], in0=mv[:, 0:1], in1=mv[:, 0:1]
            )
            nc.vector.tensor_add(
                out=mv2a[:, cc, 1:2], in0=mv2a[:, cc, 1:2], in1=mv[:, 1:2]
            )
        # aggregate to 16 group rows, then broadcast back to channels
        gp = psum_tr.tile([16, 2 * CC], F32, tag="tr", name="gp")
        nc.tensor.matmul(
            gp, lhsT=gA, rhs=mv2a.rearrange("p a b -> p (a b)"),
            start=True, stop=True,
        )
        gp_sb = work.tile([16, 2 * CC], F32, tag="gp_sb", bufs=1)
        nc.vector.tensor_copy(out=gp_sb, in_=gp)
        chs = psum_tr.tile([128, 2 * CC], F32, tag="tr", name="chs")
        nc.tensor.matmul(chs, lhsT=gB, rhs=gp_sb, start=True, stop=True)
        chs_sb = work.tile([128, CC, 2], F32, tag="chs_sb", bufs=1)
        nc.vector.tensor_copy(
            out=chs_sb.rearrange("p a b -> p (a b)"), in_=chs
        )
        mean_v = chs_sb[:, :, 0]  # [128, CC] stride 2
        ex2_v = chs_sb[:, :, 1]
        # var = E[x^2] - mean^2 ; rstd = 1/sqrt(var+eps)
        var2 = work.tile([128, CC], F32, tag="var2", bufs=1)
        msq2 = work.tile([128, CC], F32, tag="msq2", bufs=1)
        nc.vector.tensor_mul(out=msq2, in0=mean_v, in1=mean_v)
        nc.vector.tensor_sub(out=var2, in0=ex2_v, in1=msq2)
        nc.scalar.activation(
            out=var2, in_=var2,
            func=mybir.ActivationFunctionType.Sqrt, bias=eps_col,
        )
        # preload the exp act table while the ACT engine is otherwise idle
        nc.scalar.activation(
            out=scratch, in_=scratch,
            func=mybir.ActivationFunctionType.Exp, bias=ebias,
        )
        rstd2 = work.tile([128, CC], F32, tag="rstd2", bufs=1)
        nc.vector.reciprocal(out=rstd2, in_=var2)
        # a = rstd*gn_scale ; b = gn_bias - mean*a ; store (8a, b)
        a2 = work.tile([128, CC], F32, tag="a2", bufs=1)
        nc.vector.tensor_mul(out=a2, in0=rstd2, in1=gns)
        nc.vector.tensor_mul(out=msq2, in0=mean_v, in1=a2)
        nc.vector.tensor_sub(out=ab[:, :, 1], in0=gnb, in1=msq2)
        nc.vector.tensor_scalar_mul(out=ab[:, :, 0], in0=a2, scalar1=WS)

        # ---- fold the affine into fp8 qkv weights:
        # qkv8 = (8 w a)^T x8 + 8(w^T b + b_qkv) ----
        # fp8 weights + fp8 residual: the 2-term qkv matmul removes the
        # weight-quantization noise for one-time cost (16 extra matmuls)
        wq8 = consts.tile([128, CC, 3 * C], F8)
        wlo8 = consts.tile([128, CC, 3 * C], F8)
        for cc in range(CC):
            # wq8 on ACT (1-input mul), wlo8 on DVE -- runs in parallel
            nc.scalar.mul(
                out=wq8[:, cc, :], in_=wq_stage[:, cc, :],
                mul=ab[:, cc, 0:1],
            )
            nc.vector.scalar_tensor_tensor(
                out=wlo8[:, cc, :], in0=wq_stage[:, cc, :],
                scalar=ab[:, cc, 0:1], in1=wq8[:, cc, :],
                op0=mybir.AluOpType.mult, op1=mybir.AluOpType.subtract,
            )
        bias2 = work.tile([128, 6], F32, tag="bias2", bufs=1)  # 8*(w^T b + bq)
        psb = psum_tr.tile([128, 6], F32, tag="tr", name="psb")
        for m in range(6):
            for cc in range(CC):
                nc.tensor.matmul(
                    psb[:, m : m + 1],
                    lhsT=wq_stage[:, cc, m * 128 : (m + 1) * 128],
                    rhs=ab[:, cc, 1:2],
                    start=(cc == 0),
                    stop=(cc == CC - 1),
                )
        nc.vector.scalar_tensor_tensor(
            out=bias2, in0=psb, scalar=WS, in1=bq8,
            op0=mybir.AluOpType.mult, op1=mybir.AluOpType.add,
        )

        # V bias broadcast row (token-major): transpose bias2[:, 4:6] to
        # [2, 128], then two K=1 broadcasts into bv_tok [128, 256]
        # V bias folds into o_un via the colsum instead of into v_tm8:
        # o_un = av + (cs/u) * bias2_v per channel chunk, u = ONES_VAL
        bvs = work.tile([128, CC], F32, tag="bvs", bufs=1)
        nc.vector.tensor_scalar_mul(
            out=bvs, in0=bias2[:, 4:6], scalar1=1.0 / ONES_VAL
        )
        # b_proj broadcast to all partitions (token-major add at the end):
        # K=1 matmul trick, out[p, c] = ones1[0, p] * bp_row[0, c]
        bp_tok = consts.tile([128, C], F32)
        bc_ps = psum_big.tile([128, 1024], F32, tag="pair", name="bc_ps")
        nc.tensor.matmul(
            bc_ps[:, :C], lhsT=ones1, rhs=bp_row, start=True, stop=True
        )
        nc.vector.tensor_copy(out=bp_tok, in_=bc_ps[:, :C])

        # ---- phase B: qkv. K and Q channel-major fp8 with fp8 RESIDUALS
        # (3-term logits halve the attention-weight error); V directly
        # token-major: v_tm8[nt] = x_cm8[:, :, nt].T @ wv8 ----
        qkv8 = qkv8_pool.tile([128, 4, N], F8)  # Q chunks 0-1, K chunks 2-3
        v_tm8 = vtm8_pool.tile([128, 32, C], F8)

        def emit_qkv(m, qt, i):
            ps = psum_big.tile([128, 1024], F32, tag="pair", name="qkv_ps")
            for wmat, st in ((wq8, True), (wlo8, False)):
                nc.tensor.matmul(
                    ps[:, :QT],
                    lhsT=wmat[:, :, m * 128 : (m + 1) * 128],
                    rhs=x_cm8[:, :, qt * QT : (qt + 1) * QT],
                    start=st, stop=not st, perf_mode=DR,
                )
            nc.vector.tensor_scalar_add(
                out=qkv8[:, m, qt * QT : (qt + 1) * QT],
                in0=ps[:, :QT], scalar1=bias2[:, m : m + 1],
            )

        def emit_k2(m, g):
            """K for token blocks 2g,2g+1: two bank-sized matmuls sharing one
            wide [128,1024] drain."""
            ps = psum_big.tile([128, 1024], F32, tag="pair", name="k_ps")
            for h in range(2):
                for wmat, st in ((wq8, True), (wlo8, False)):
                    nc.tensor.matmul(
                        ps[:, h * QT : (h + 1) * QT],
                        lhsT=wmat[:, :, m * 128 : (m + 1) * 128],
                        rhs=x_cm8[:, :, (2 * g + h) * QT : (2 * g + h + 1) * QT],
                        start=st, stop=not st, perf_mode=DR,
                    )
            nc.vector.tensor_scalar_add(
                out=qkv8[:, m, g * 1024 : (g + 1) * 1024],
                in0=ps, scalar1=bias2[:, m : m + 1],
            )

        def emit_v4(nt0):
            """v for 4 token tiles in one slot: 4 matmuls + one wide drain.
            V bias is folded later via the colsum, so this is a plain copy."""
            ps = psum_big.tile([128, 1024], F32, tag="pair", name="v_ps")
            for i in range(4):
                # V stays 1-term: its weight-quantization noise averages out
                # across the softmax sum
                nc.tensor.matmul(
                    ps[:, i * 256 : (i + 1) * 256],
                    lhsT=x_cm8[:, :, (nt0 + i) * 128 : (nt0 + i + 1) * 128],
                    rhs=wq8[:, :, 4 * 128 : 6 * 128],
                    start=True, stop=True, perf_mode=DR,
                )
            nc.vector.tensor_copy(out=v_tm8[:, nt0 : nt0 + 4, :], in_=ps)

        # minimal pre-attention set: K(token blocks 0-3), Q(qt0), v(0..7).
        # The rest streams inside the qt0 pair loop so the psum pair-ring
        # FIFO pipelines qkv drains with logits+exp instead of serializing.
        emit_k2(2, 0)
        emit_k2(3, 0)
        emit_qkv(0, 0, 0)
        emit_qkv(1, 0, 1)
        emit_v4(0)
        emit_v4(4)

        # ---- phase D: attention + proj + skip, per q tile ----
        def emit_lg_pair(qt, j):
            """Two logits tiles into one 2-bank psum slot + their exp."""
            pair = psum_big.tile([128, 1024], F32, tag="pair", name="lg")
            qhi = qkv8[:, 0:2, qt * QT : (qt + 1) * QT]
            for j2 in range(2):
                kt = 2 * j + j2
                nc.tensor.matmul(
                    pair[:, j2 * QT : (j2 + 1) * QT],
                    lhsT=qkv8[:, 2:4, kt * 128 : (kt + 1) * 128],
                    rhs=qhi, start=True, stop=True, perf_mode=DR,
                )
            e8 = expp.tile([128, 2, QT], F8, tag="e8")
            nc.scalar.activation(
                out=e8.rearrange("p g q -> p (g q)"), in_=pair,
                func=mybir.ActivationFunctionType.Exp,
                scale=1.0 / (16.0 * WS * WS), bias=ebias,
            )
            return e8

        pre = [emit_lg_pair(0, 0), emit_lg_pair(0, 1)]
        for qt in range(NQ):
            # fold b_proj into the skip source for this q tile (in-place)
            for qq in range(4):
                nc.gpsimd.tensor_add(
                    out=x_tm[:, qt * 4 + qq, :], in0=x_tm[:, qt * 4 + qq, :],
                    in1=bp_tok,
                )
            av2 = psum_acc.tile([128, CC, QT], F32, tag="av", name="av")
            av_ps = [av2[:, cc, :] for cc in range(CC)]
            cs_ps = psum_acc.tile([128, QT], F32, tag="cs", name="cs")
            for j in range(NPAIR):
                if qt == 0:
                    # stream K token-blocks ahead of their logits use and v
                    # tiles ahead of their av use
                    if j % 4 == 0 and j <= 8:
                        emit_k2(2, j // 4 + 1)
                        emit_k2(3, j // 4 + 1)
                    if j % 2 == 0 and j <= 10:
                        emit_v4(8 + 2 * j)
                if j == 11 and qt < NQ - 1:
                    # next q tile's Q chunks (consumed by the prefetch pairs)
                    emit_qkv(0, qt + 1, j)
                    emit_qkv(1, qt + 1, j + 1)
                e8 = pre[j] if j < len(pre) else emit_lg_pair(qt, j)
                for cc in range(CC):
                    nc.tensor.matmul(
                        av_ps[cc],
                        lhsT=v_tm8[:, 2 * j : 2 * j + 2, cc * 128 : (cc + 1) * 128],
                        rhs=e8,
                        start=(j == 0), stop=(j == NPAIR - 1),
                        perf_mode=DR,
                    )
                nc.tensor.matmul(
                    cs_ps, lhsT=ones8, rhs=e8,
                    start=(j == 0), stop=(j == NPAIR - 1), perf_mode=DR,
                )

            # prefetch the next q tile's first pairs so neither the ACT exp
            # stream nor the PE waits on the boundary drain chain below
            pre = (
                [emit_lg_pair(qt + 1, jj) for jj in range(4)]
                if qt + 1 < NQ else []
            )
            recip = work.tile([128, QT], F32, tag="recip")
            nc.vector.reciprocal_approx_fast(out=recip, in_=cs_ps)
            # o_un = av + (cs/u)*bias2_v : folds the V bias in (two ops per
            # chunk -- the hardware allows only one PSUM input per op)
            o_un = work.tile([128, CC, QT], F32, tag="o_un")
            for cc in range(CC):
                nc.vector.tensor_copy(
                    out=o_un[:, cc, :].bitcast(F32R), in_=av_ps[cc]
                )
                nc.vector.scalar_tensor_tensor(
                    out=o_un[:, cc, :].bitcast(F32R), in0=cs_ps,
                    scalar=bvs[:, cc : cc + 1], in1=o_un[:, cc, :],
                    op0=mybir.AluOpType.mult, op1=mybir.AluOpType.add,
                )
            # proj in fp32r: pj = wp^T o_un ; * recip -> bf16 for the
            # transpose. pj reuses the cs/av banks (ring-1 deps = exactly the
            # recip/o_un reads) so the pair ring stays free for the next q
            # tile's prefetched logits.
            pjm = work.tile([128, CC, QT], BF16, tag="pjm")
            for dc in range(CC):
                if dc == 0:
                    ps = psum_acc.tile([128, QT], F32, tag="cs", name="pj_cs")
                else:
                    ps = psum_acc.tile(
                        [128, CC, QT], F32, tag="av", name="pj_av"
                    )[:, 0, :]
                for cc in range(CC):
                    nc.tensor.matmul(
                        ps,
                        lhsT=wp[:, cc, dc * 128 : (dc + 1) * 128].bitcast(F32R),
                        rhs=o_un[:, cc, :].bitcast(F32R),
                        start=(cc == 0), stop=(cc == CC - 1),
                    )
                nc.vector.tensor_mul(
                    out=pjm[:, dc, :], in0=ps, in1=recip
                )
            # back to token-major; add skip (+b_proj already folded into x_tm)
            out_sb = work.tile([128, 4, C], F32, tag="out_sb")
            for dc in range(CC):
                trs = psum_tr.tile([128, 4, 128], BF16, tag="tr", name="ps_out")
                for qq in range(4):
                    nc.tensor.transpose(
                        trs[:, qq, :], pjm[:, dc, qq * 128 : (qq + 1) * 128],
                        ident16,
                    )
                nc.vector.tensor_add(
                    out=out_sb[:, :, dc * 128 : (dc + 1) * 128],
                    in0=trs,
                    in1=x_tm[:, qt * 4 : (qt + 1) * 4, dc * 128 : (dc + 1) * 128],
                )
            nc.sync.dma_start(
                out=out_tok[:, qt * 4 : (qt + 1) * 4, :], in_=out_sb
            )


_NC = None


def _get_nc():
    global _NC
    if _NC is None:
        _NC = _build()
    return _NC


_RUNNER = None
_ZEROS_FN = None

IN_NAMES = ["x", "gn_scale", "gn_bias", "w_qkv", "b_qkv", "w_proj", "b_proj"]


def _get_runner():
    """Cached jitted shard_map executable over the 8 cores (the equivalent of
    run_bass_kernel_spmd's axon path, but built once instead of per call)."""
    global _RUNNER
    if _RUNNER is not None:
        return _RUNNER
    import jax
    from jax.sharding import Mesh, PartitionSpec
    from jax.experimental.shard_map import shard_map
    from concourse import bass2jax

    nc = _get_nc()
    bass2jax.install_neuronx_cc_hook()

    in_names = list(IN_NAMES) + ["out"]
    if nc.partition_id_tensor is not None:
        in_names.append(nc.partition_id_tensor.name)

    def _body_fn(*args):
        operands = list(args)
        if nc.partition_id_tensor is not None:
            operands.append(bass2jax.partition_id_tensor())
        outs = bass2jax._bass_exec_p.bind(
            *operands,
            out_avals=(jax.core.ShapedArray((N, C), np.float32),),
            in_names=tuple(in_names),
            out_names=("out",),
            lowering_input_output_aliases=(),
            sim_require_finite=True,
            sim_require_nnan=True,
            nc=nc,
        )
        return tuple(outs)

    devices = jax.devices()[:N_CORES]
    mesh = Mesh(np.asarray(devices), ("core",))
    in_specs = (PartitionSpec("core"),) * (len(IN_NAMES) + 1)
    out_specs = (PartitionSpec("core"),)
    sharded = jax.jit(
        shard_map(
            _body_fn, mesh=mesh, in_specs=in_specs, out_specs=out_specs,
            check_rep=False,
        ),
        donate_argnums=(len(IN_NAMES),),
        keep_unused=True,
    )
    _RUNNER = sharded
    return _RUNNER


def kernel(x, gn_scale, gn_bias, w_qkv, b_qkv, w_proj, b_proj):
    sharded = _get_runner()
    x = np.ascontiguousarray(np.asarray(x, dtype=np.float32).reshape(B * N, C))
    shared = {
        "gn_scale": np.asarray(gn_scale, np.float32),
        "gn_bias": np.asarray(gn_bias, np.float32),
        "w_qkv": np.ascontiguousarray(np.asarray(w_qkv, np.float32)),
        "b_qkv": np.asarray(b_qkv, np.float32),
        "w_proj": np.ascontiguousarray(np.asarray(w_proj, np.float32)),
        "b_proj": np.asarray(b_proj, np.float32),
    }
    # shard_map slices axis 0 across cores: x gets its own batch; the shared
    # weights are tiled 8x so every core sees an identical copy.
    concat = [x]
    for name in IN_NAMES[1:]:
        a = shared[name]
        concat.append(np.concatenate([a] * N_CORES, axis=0))
    # donated output buffer, created on-device (saves a 32MB host->device
    # transfer through the axon tunnel every call)
    import jax
    import jax.numpy as jnp
    from jax.sharding import Mesh, NamedSharding, PartitionSpec

    global _ZEROS_FN
    if _ZEROS_FN is None:
        mesh = Mesh(np.asarray(jax.devices()[:N_CORES]), ("core",))
        sh = NamedSharding(mesh, PartitionSpec("core"))
        _ZEROS_FN = jax.jit(
            lambda: jnp.zeros((N_CORES * N, C), jnp.float32), out_shardings=sh
        )
    zeros = _ZEROS_FN()
    (out,) = sharded(*concat, zeros)
    return np.asarray(out).reshape(B, H, W, C)
